# revision 15
# baseline (speedup 1.0000x reference)
"""CrossAttention Trainium2 kernel (8 NeuronCores, SPMD, tensor-parallel).

Sharding per the TP hint: 8 cores = batch(2) x head-group(4 x 4 heads).
Each core projects q/k/v for its 256-wide slice of the cross-attention dim
(Wq/Wk/Wv column shards), runs attention for its 4 heads over the full
4096-token query range of its batch, and multiplies by its 256-row shard
of Wo, producing a PARTIAL [4096, 1024] output. The host unshards by
summing the 4 partials per batch (f32).

This removes the 4x duplicated K/V projection work of query-block
sharding; no device collectives are needed.

Per-core tensor layout is feature-major ([feat, token]); local heads
h = 2*p + j for feature-block p in {0,1}, j in {0,1}.
rstd = exp(-0.5*ln(ssq/64+eps)) keeps everything in one ACT table set;
softmax reciprocals via fast DVE Newton-Raphson from the PSUM sum row.
"""

import numpy as np
import ml_dtypes
from contextlib import ExitStack

import concourse.bass as bass
import concourse.tile as tile
from concourse import bacc, mybir
from concourse.bass_utils import run_bass_kernel_spmd

BF = mybir.dt.bfloat16
F32 = mybir.dt.float32
F32R = mybir.dt.float32r

D = 1024      # model dim
H = 16        # total heads
HL = 4        # heads per core
FL = HL * 64  # local cross-attention features (256)
HD = 64       # head dim
MQ = 4096     # query tokens per core (full batch row)
SKV = 1024    # kv tokens
B = 2
NCORES = 8
LN_EPS = 1e-5
CH = 512      # query-token chunk
NQC = MQ // CH  # 8 query chunks


# Force a single ACT table set (covers Copy/Exp/Ln) so interleaved Ln/Exp
# activations never thrash table loads: mask every other set's functions.
import concourse.bacc as _bacc_mod
import concourse.bass_interp as _interp_mod
from concourse import hw_specs as _hw_specs


def _ln_exp_only_tables(arch):
    tabs = _hw_specs.get_activation_tables(arch)
    return {k: (v if k == "natural_log_exp_and_others" else set())
            for k, v in tabs.items()}


_bacc_mod.get_activation_tables = _ln_exp_only_tables
_interp_mod.get_activation_tables = _ln_exp_only_tables

_cache = {}


def _selector_constants():
    # sel4[d][p, j]: 1 if local head j == 2d + p//64
    sel4 = np.zeros((2, 128, HL), np.float32)
    for d in range(2):
        for p in range(128):
            sel4[d, p, 2 * d + p // 64] = 1.0
    selB4 = np.transpose(sel4, (0, 2, 1)).copy()
    selE = np.eye(HL, dtype=np.float32)[None, :, :]
    return sel4, selB4, selE


def _emit(ctx: ExitStack, tc, t, has_bias_q, has_bias_k):
    nc = tc.nc

    persist = ctx.enter_context(tc.tile_pool(name="persist", bufs=1))

    # ---- persistent SBUF tensors ----
    hst = persist.tile([128, 8, MQ], BF, tag="hst")      # hs^T full batch row
    enct = persist.tile([128, 8, SKV], BF, tag="enct")   # enc^T
    wq = persist.tile([128, 8, FL], BF, tag="wq")        # [k-block, out-feat]
    wk = persist.tile([128, 8, FL], BF, tag="wk")
    wv = persist.tile([128, 8, FL], BF, tag="wv")
    wo = persist.tile([128, 2, D], BF, tag="wo")         # [row-block, out-col]
    qtln = persist.tile([128, 2, MQ], BF, tag="qtln")    # LN(q)^T
    ktln = persist.tile([128, 2, SKV], BF, tag="ktln")   # LN(k)^T
    vaug = persist.tile([128, 8, HL, HD + 1], BF, tag="vaug")  # [kv, h, V|1]
    aout = persist.tile([128, 2, MQ], BF, tag="aout")    # attn out^T
    gq_sb = persist.tile([128, 2], F32, tag="gq_sb")
    gk_sb = persist.tile([128, 2], F32, tag="gk_sb")
    sel4_sb = persist.tile([128, 2, HL], BF, tag="sel4_sb")
    selB_sb = persist.tile([HL, 2, 128], BF, tag="selB_sb")
    selE_sb = persist.tile([1, HL, HL], BF, tag="selE_sb")
    rinv_q = persist.tile([HL, MQ], BF, tag="rinv_q")
    rinv_k = persist.tile([HL, SKV], BF, tag="rinv_k")
    inv_s = persist.tile([HL, MQ], BF, tag="inv_s")
    sinv = persist.tile([HL, 512], F32, tag="sinv")
    eps_sb = persist.tile([HL, 1], F32, tag="eps_sb")
    nc.vector.memset(eps_sb[:, :], LN_EPS)
    nc.vector.memset(vaug[:, :, :, HD:HD + 1], 1.0)
    bq_sb = persist.tile([128, 2], F32, tag="bq_sb") if has_bias_q else None
    bk_sb = persist.tile([128, 2], F32, tag="bk_sb") if has_bias_k else None

    # ---- loads (K path first so the K projection starts immediately) ----
    for k in range(8):
        nc.sync.dma_start(enct[:, k, :], t["encT"][k * 128:(k + 1) * 128, :])
        nc.sync.dma_start(wk[:, k, :], t["wk"][k * 128:(k + 1) * 128, :])
    for k in range(8):
        nc.sync.dma_start(wq[:, k, :], t["wq"][k * 128:(k + 1) * 128, :])
        nc.sync.dma_start(wv[:, k, :], t["wv"][k * 128:(k + 1) * 128, :])
    for k in range(8):
        nc.sync.dma_start(hst[:, k, :], t["hsT"][k * 128:(k + 1) * 128, :])
    for k in range(2):
        nc.sync.dma_start(wo[:, k, :], t["wo"][k * 128:(k + 1) * 128, :])
    nc.sync.dma_start(gq_sb[:, :], t["gq"].rearrange("(d p) -> p d", p=128))
    nc.sync.dma_start(gk_sb[:, :], t["gk"].rearrange("(d p) -> p d", p=128))
    if has_bias_q:
        nc.sync.dma_start(bq_sb[:, :], t["bq"].rearrange("(d p) -> p d", p=128))
    if has_bias_k:
        nc.sync.dma_start(bk_sb[:, :], t["bk"].rearrange("(d p) -> p d", p=128))
    nc.sync.dma_start(sel4_sb[:, :, :], t["sel4"].rearrange("d p j -> p d j"))
    nc.sync.dma_start(selB_sb[:, :, :], t["selB"].rearrange("d j p -> j d p"))
    nc.sync.dma_start(selE_sb[:, :, :], t["selE"])

    sq_pool = ctx.enter_context(tc.tile_pool(name="sq_pool", bufs=2))
    lnt_pool = ctx.enter_context(tc.tile_pool(name="lnt_pool", bufs=2))
    ps_aux = ctx.enter_context(tc.tile_pool(name="ps_aux", bufs=1, space="PSUM"))

    def proj_ln(ps_proj, w_sb, x_sb, ln_sb, g_sb, b_sb, rinv_sb, chunks,
                stage_act=False):
        # project 512-token chunk c: ln_sb[:, d, c*512:...] for d in {0,1},
        # per-head LN stats via selector matmuls
        for c in chunks:
            ssq = ps_aux.tile([HL, 512], F32, tag="aux", name=f"ssq{c % 2}")
            for d in range(2):
                acc = ps_proj.tile([128, 512], F32, tag="acc", name=f"acc{d}")
                for k in range(8):
                    nc.tensor.matmul(
                        acc[:, :],
                        lhsT=w_sb[:, k, d * 128:(d + 1) * 128],
                        rhs=x_sb[:, k, c * 512:(c + 1) * 512],
                        start=(k == 0), stop=(k == 7),
                    )
                if stage_act:
                    nc.scalar.copy(ln_sb[:, d, c * 512:(c + 1) * 512],
                                   acc[:, :])
                else:
                    nc.vector.tensor_copy(
                        ln_sb[:, d, c * 512:(c + 1) * 512], acc[:, :])
                sq = sq_pool.tile([128, 512], BF)
                nc.vector.tensor_mul(sq[:, :],
                                     ln_sb[:, d, c * 512:(c + 1) * 512],
                                     ln_sb[:, d, c * 512:(c + 1) * 512])
                nc.tensor.matmul(
                    ssq[:, :],
                    lhsT=sel4_sb[:, d, :],
                    rhs=sq[:, :],
                    start=(d == 0), stop=(d == 1),
                    skip_group_check=True,
                )
            # rinv = (ssq/64+eps)^-1/2 = exp(-0.5*ln(ssq/64+eps))
            lnt = lnt_pool.tile([HL, 512], F32)
            nc.scalar.activation(
                lnt[:, :], ssq[:, :], mybir.ActivationFunctionType.Ln,
                bias=eps_sb[:, :], scale=1.0 / HD,
            )
            nc.scalar.activation(
                rinv_sb[:, c * 512:(c + 1) * 512], lnt[:, :],
                mybir.ActivationFunctionType.Exp, scale=-0.5,
            )
            for d in range(2):
                rb = ps_aux.tile([128, 512], F32, tag="aux", name="rb")
                nc.tensor.matmul(
                    rb[:, :],
                    lhsT=selB_sb[:, d, :],
                    rhs=rinv_sb[:, c * 512:(c + 1) * 512],
                    start=True, stop=True,
                )
                dst = ln_sb[:, d, c * 512:(c + 1) * 512]
                nc.vector.scalar_tensor_tensor(
                    out=dst,
                    in0=dst,
                    scalar=g_sb[:, d:d + 1],
                    in1=rb[:, :],
                    op0=mybir.AluOpType.mult,
                    op1=mybir.AluOpType.mult,
                )
                if b_sb is not None:
                    nc.vector.tensor_scalar_add(dst, dst, b_sb[:, d:d + 1])

    # ---- head phase: K proj (2 chunks), V proj, Q proj chunk 0 ----
    with tc.tile_pool(name="ps_head", bufs=4, space="PSUM") as ps_head:
        proj_ln(ps_head, wk, enct, ktln, gk_sb, bk_sb, rinv_k, [0, 1],
                stage_act=True)
        # V projection into augmented layout [kv, h, V|1]
        for tt in range(8):
            acc = ps_head.tile([128, FL], F32, tag="acc", name="vacc")
            for k in range(8):
                nc.tensor.matmul(
                    acc[:, :],
                    lhsT=enct[:, k, tt * 128:(tt + 1) * 128],
                    rhs=wv[:, k, :],
                    start=(k == 0), stop=(k == 7),
                )
            dst = vaug[:, tt, :, 0:HD]
            nc.scalar.copy(
                dst, acc[:, :].rearrange("p (h e) -> p h e", e=HD))
        proj_ln(ps_head, wq, hst, qtln, gq_sb, bq_sb, rinv_q, [0])

    # steady state: PSUM = acc(1) + aux(1) + sc(2x2) + av(2) = 8 banks
    ps_tail = ctx.enter_context(tc.tile_pool(name="ps_tail", bufs=1,
                                             space="PSUM"))
    at_pool = ctx.enter_context(tc.tile_pool(name="at_pool", bufs=4))
    out_pool = ctx.enter_context(tc.tile_pool(name="out_pool", bufs=4))
    srow_pool = ctx.enter_context(tc.tile_pool(name="srow_pool", bufs=2))
    ps_sc = ctx.enter_context(tc.tile_pool(name="ps_sc", bufs=2, space="PSUM"))
    ps_av = ctx.enter_context(tc.tile_pool(name="ps_av", bufs=2, space="PSUM"))

    def attn_chunk(c):
        sums = ps_av.tile([HL, CH], F32, tag="av", name="sums")
        for p in range(2):
            av2 = ps_av.tile([HD + 1, 2, CH], F32, tag="av", name="av2")
            for quarter in range(4):
                scs = {j: ps_sc.tile([128, 2, CH], F32, tag="sc",
                                     name=f"sc{j}") for j in range(2)}
                for vv in range(2):
                    v = 2 * quarter + vv
                    for j in range(2):
                        nc.tensor.matmul(
                            scs[j][:, vv, :],
                            lhsT=ktln[j * 64:(j + 1) * 64, p,
                                      v * 128:(v + 1) * 128],
                            rhs=qtln[j * 64:(j + 1) * 64, p,
                                     c * CH:(c + 1) * CH],
                            start=True, stop=True,
                        )
                for j in range(2):
                    at = at_pool.tile([128, 2, CH], BF)
                    nc.scalar.activation(
                        at[:, :, :], scs[j][:, :, :],
                        mybir.ActivationFunctionType.Exp, scale=0.125,
                    )
                    for vv in range(2):
                        v = 2 * quarter + vv
                        nc.tensor.matmul(
                            av2[:, j, :],
                            lhsT=vaug[:, v, 2 * p + j, :],
                            rhs=at[:, vv, :],
                            start=(v == 0), stop=(v == 7),
                            skip_group_check=True,
                        )
            for j in range(2):
                h = 2 * p + j
                nc.vector.tensor_copy(
                    aout[j * 64:(j + 1) * 64, p, c * CH:(c + 1) * CH],
                    av2[0:HD, j, :])
                srow = srow_pool.tile([1, CH], BF)
                nc.vector.tensor_copy(srow[:, :], av2[HD:HD + 1, j, :])
                nc.tensor.matmul(
                    sums[:, :],
                    lhsT=selE_sb[:, h, :],
                    rhs=srow[:, :],
                    start=(h == 0), stop=(h == 3),
                    skip_group_check=True,
                )
        # softmax denominators: fast NR reciprocal on the gathered sums
        nc.vector.reciprocal_approx_fast(out=sinv[:, :], in_=sums[:, :])
        nc.vector.tensor_copy(inv_s[:, c * CH:(c + 1) * CH], sinv[:, :])

    def norm_out_chunk(c):
        for p in range(2):
            rb = ps_aux.tile([128, CH], F32, tag="aux", name="rb2")
            nc.tensor.matmul(
                rb[:, :],
                lhsT=selB_sb[:, p, :],
                rhs=inv_s[:, c * CH:(c + 1) * CH],
                start=True, stop=True,
            )
            sl = aout[:, p, c * CH:(c + 1) * CH]
            nc.vector.tensor_mul(sl, sl, rb[:, :])

        for tt in range(4 * c, 4 * (c + 1)):
            for cc in range(2):
                acc = ps_tail.tile([128, 512], F32, tag="acc", name="oacc")
                for k in range(2):
                    nc.tensor.matmul(
                        acc[:, :],
                        lhsT=aout[:, k, tt * 128:(tt + 1) * 128],
                        rhs=wo[:, k, cc * 512:(cc + 1) * 512],
                        start=(k == 0), stop=(k == 1),
                    )
                ot = out_pool.tile([128, 512], BF)
                nc.scalar.copy(ot[:, :], acc[:, :])
                nc.sync.dma_start(
                    t["out"][tt * 128:(tt + 1) * 128,
                             cc * 512:(cc + 1) * 512],
                    ot[:, :],
                )

    # steady-state pipeline over 8 query chunks: Q proj of chunk c+1 and
    # Wo projection of chunk c-1 fill PE gaps while ACT exps chunk c
    attn_chunk(0)
    for c in range(NQC):
        if c + 1 < NQC:
            proj_ln(ps_tail, wq, hst, qtln, gq_sb, bq_sb, rinv_q, [c + 1])
        if c > 0:
            norm_out_chunk(c - 1)
        if c + 1 < NQC:
            attn_chunk(c + 1)
    norm_out_chunk(NQC - 1)


def _build(has_bias_q, has_bias_k):
    key = (has_bias_q, has_bias_k)
    if key in _cache:
        return _cache[key]
    nc = bacc.Bacc("TRN2", target_bir_lowering=False, debug=False,
                   num_devices=NCORES)
    t = {}

    def inp(name, shape, dt):
        t[name] = nc.dram_tensor(name, list(shape), dt, kind="ExternalInput").ap()

    inp("hsT", (D, MQ), BF)
    inp("encT", (D, SKV), BF)
    inp("wq", (D, FL), BF)
    inp("wk", (D, FL), BF)
    inp("wv", (D, FL), BF)
    inp("wo", (FL, D), BF)
    inp("gq", (FL,), F32)
    inp("gk", (FL,), F32)
    if has_bias_q:
        inp("bq", (FL,), F32)
    if has_bias_k:
        inp("bk", (FL,), F32)
    inp("sel4", (2, 128, HL), BF)
    inp("selB", (2, HL, 128), BF)
    inp("selE", (1, HL, HL), BF)
    t["out"] = nc.dram_tensor("out", [MQ, D], BF, kind="ExternalOutput").ap()

    with tile.TileContext(nc) as tc:
        with ExitStack() as ctx:
            _emit(ctx, tc, t, has_bias_q, has_bias_k)
    nc.finalize()
    _cache[key] = nc
    return nc


def _center_fold(W):
    # Fold per-head output-column mean removal into the weight matrix (exact).
    Wr = np.asarray(W, np.float32).reshape(D, H, HD)
    return (Wr - Wr.mean(axis=2, keepdims=True)).reshape(D, D)


def kernel(hidden_states, encoder_hidden_states, Wq, Wk, Wv, Wo,
           gq, bq, gk, bk, _trace=False):
    hs = np.asarray(hidden_states, np.float32)
    enc = np.asarray(encoder_hidden_states, np.float32)
    bq = np.asarray(bq, np.float32)
    bk = np.asarray(bk, np.float32)
    has_bias_q = bool(np.any(bq != 0))
    has_bias_k = bool(np.any(bk != 0))
    nc = _build(has_bias_q, has_bias_k)

    bf = ml_dtypes.bfloat16
    wq_f = _center_fold(Wq)
    wk_f = _center_fold(Wk)
    wv_f = np.asarray(Wv, np.float32)
    wo_f = np.asarray(Wo, np.float32)
    gq_rep = np.tile(np.asarray(gq, np.float32), HL)
    gk_rep = np.tile(np.asarray(gk, np.float32), HL)
    sel4, selB4, selE = _selector_constants()

    in_maps = []
    for core in range(NCORES):
        b, hg = divmod(core, 4)
        fs = slice(hg * FL, (hg + 1) * FL)
        m = {
            "hsT": np.ascontiguousarray(hs[b].T).astype(bf),
            "encT": np.ascontiguousarray(enc[b].T).astype(bf),
            "wq": np.ascontiguousarray(wq_f[:, fs]).astype(bf),
            "wk": np.ascontiguousarray(wk_f[:, fs]).astype(bf),
            "wv": np.ascontiguousarray(wv_f[:, fs]).astype(bf),
            "wo": np.ascontiguousarray(wo_f[fs, :]).astype(bf),
            "gq": gq_rep, "gk": gk_rep,
            "sel4": sel4.astype(bf), "selB": selB4.astype(bf),
            "selE": selE.astype(bf),
        }
        if has_bias_q:
            m["bq"] = np.tile(bq, HL)
        if has_bias_k:
            m["bk"] = np.tile(bk, HL)
        in_maps.append(m)

    res = run_bass_kernel_spmd(nc, in_maps, list(range(NCORES)), trace=_trace)

    out = np.zeros((B, MQ, D), np.float32)
    for core in range(NCORES):
        b, _ = divmod(core, 4)
        out[b] += np.asarray(res.results[core]["out"], np.float32)
    kernel.last_exec_time_ns = res.exec_time_ns
    kernel.last_results = res
    return out
